# revision 7
# baseline (speedup 1.0000x reference)
"""CBOW negative-sampling loss kernel for Trainium2 (8 cores, Bass/Tile).

Architecture: window-sorted dma_gather -> parity-split CCE dma_scatter_add
into canonical per-(item, role) SBUF slots -> DVE reduction tail (ctx
tree-add, broadcast-mult-reduce scores, 4th-order Taylor softplus),
data-parallel over the batch (16384 -> 8 x 2048 items), tables replicated.

Performance notes (vs the 524us single-queue v1):
- num_swdge_queues=4 with item-quarter q scattering on queue q: the four
  disjoint CCE RMW chains drain on four descriptor rings concurrently.
  The single-queue version was ring-drain bound at ~9 ns/descriptor.
- single_packet=False on scatters: packet concat amortizes the SDMA
  m2s<->s2m bus overhead (~27% for 256B descriptors).
- Pad tokens spread across the 256 dustbin slots (2 ranks x 128
  partitions): a single shared dustbin slot serialized same-address RMWs.
- Tables converted to bf16 on host and gathered directly (elem 256B):
  numerically identical to f32-gather + bf16-cast, one less stage.
- Gathers merged per (stream, window) and chunked at 1024 rows (the
  per-direction SWDGE ring capacity; larger calls hang the ucode).
- Balanced 25000-row vocab windows (int16 idx limit allows <= 32768).
- Tail overlap: ctx reductions emit right after the last ctx scatter
  group; per-quarter score reductions interleave with the final out
  group's scatters, so only the last quarter's reduce is exposed.

Spare slot (j=11 of OUT_SLOTS=12) stays zero and contributes exactly ln2
per item, subtracted on host. Window caps are exact per-input (max over
cores, 128-aligned); the program cache is keyed by the cap tuple.
"""
import sys

if '/opt/trn_rl_repo' not in sys.path:
    sys.path.insert(0, '/opt/trn_rl_repo')

import numpy as np

P = 128          # partitions
D = 128          # embedding dim
CTX = 10         # context window
NOUT = 11        # 1 target + 10 negatives
OUT_SLOTS = 12   # 11 roles + 1 spare (even slot count for parity split)
V = 100000       # vocab rows
WB = 25000       # balanced window size (< 32768 int16 gather limit)
NW = 4
NCORES = 8
NQ = 4           # item quarters == SWDGE queues
RING = 1024      # max rows per gather/scatter call (desc-ring bound)
SCRATCH = 16384  # bytes/partition of SWDGE descriptor carveout

_PROGRAM_CACHE = {}
LN2 = float(np.log(np.float32(2.0)))
PARTS = frozenset(('gather', 'scatter', 'tail'))  # ablation control
SINGLE_PACKET = False


def _round_up(x, m):
    return ((x + m - 1) // m) * m


def _chunks_of(n):
    """Split n rows into balanced 128-aligned chunks of <= RING rows."""
    if n == 0:
        return []
    k = (n + RING - 1) // RING
    c = _round_up((n + k - 1) // k, P)
    out = []
    left = n
    while left > 0:
        take = min(c, left)
        out.append(take)
        left -= take
    return out


def _wrap_idx(vals):
    """dma_gather/scatter idx layout: idx[i] read from [i%16, i//16];
    replicate to 128 partitions."""
    n = len(vals)
    assert n % 16 == 0
    arr = np.asarray(vals, np.int16).reshape(n // 16, 16).T
    return np.tile(arr, (8, 1))


class _Plan:
    """Static layout for T*128 items/core.

    caps[s][w][q]: 128-aligned per (stream, window, quarter) capacities
    (max over cores). Gather groups span quarters: group (s, w) has
    rows [sum(caps[s][w][:q]) ... ] per quarter, total gn[s][w].
    """

    def __init__(self, T, ctx_caps, out_caps):
        assert T % NQ == 0
        self.T = T
        self.TH = T // NQ
        self.items = T * P
        self.caps = (tuple(tuple(int(c) for c in w) for w in ctx_caps),
                     tuple(tuple(int(c) for c in w) for w in out_caps))
        # group (s, w): total rows and quarter offsets
        self.gn = [[sum(self.caps[s][w]) for w in range(NW)]
                   for s in range(2)]
        self.qoff = [[tuple(np.cumsum((0,) + self.caps[s][w][:-1]))
                      for w in range(NW)] for s in range(2)]
        # flat idx offsets per (s, w) group start
        self.goff = [[sum(self.gn[s][:w]) for w in range(NW)]
                     for s in range(2)]
        self.tot = [sum(self.gn[s]) for s in range(2)]
        self.maxg = max(max(self.gn[0]), max(self.gn[1]))
        # canonical slot-rank spaces per quarter (slot = part + P*rank).
        # ctx: ONE slot per item (CCE accumulates the context sum); rank =
        # local tile, so parity alternates by tile (TH must be even).
        assert self.TH % 2 == 0
        self.r_dust_ctx = self.TH
        self.g_ctx = self.r_dust_ctx // 2 + 1
        self.r_dust_out = OUT_SLOTS * self.TH
        self.g_out = self.r_dust_out // 2 + 1
        self.s_dust_ctx = P * self.r_dust_ctx
        self.s_dust_out = P * self.r_dust_out
        self.spare_per_core = self.items


def _qw_counts(ids, items, TH):
    """ids [items, k] -> per (window, quarter) draw counts."""
    w_of = np.asarray(ids, np.int64) // WB
    qtr = (np.arange(items) // P // TH)[:, None]
    out = np.zeros((NW, NQ), np.int64)
    for w in range(NW):
        for q in range(NQ):
            out[w, q] = np.sum((w_of == w) & (qtr == q))
    return out


def _host_prep_core(plan, ctx_ids, tgt_ids, neg_ids):
    """Per-core gather/scatter int16 index tensors, (w, q)-grouped."""
    TH = plan.TH
    item_idx = np.arange(plan.items)
    tile_i = item_idx // P
    part = item_idx % P
    qtr = tile_i // TH
    ltile = tile_i - qtr * TH

    def build(ids, slot_of, caps, dust, cmajor=False):
        if cmajor:
            ids = np.ascontiguousarray(ids.T)
            slot_of = np.ascontiguousarray(slot_of.T)
            q2 = np.broadcast_to(qtr[None, :], ids.shape)
        else:
            q2 = np.broadcast_to(qtr[:, None], ids.shape)
        w_of = ids // WB
        rel = ids - w_of * WB
        gs, ss = [], []
        for w in range(NW):
            for q in range(NQ):
                sel = (w_of == w) & (q2 == q)
                g = rel[sel]
                s = slot_of[sel]
                cap = caps[w][q]
                if len(g) > cap:
                    raise RuntimeError(
                        f"window {w} quarter {q}: {len(g)} > {cap}")
                pad = cap - len(g)
                gs.append(np.concatenate([g, np.zeros(pad, np.int64)]))
                dusts = dust + (np.arange(pad) % (2 * P))
                ss.append(np.concatenate([s, dusts]))
        return (_wrap_idx(np.concatenate(gs)), _wrap_idx(np.concatenate(ss)))

    ids = np.asarray(ctx_ids, np.int64)
    slot = np.broadcast_to(
        (part + P * ltile)[:, None], ids.shape)
    ctx_g, ctx_s = build(ids, slot, plan.caps[0], plan.s_dust_ctx,
                         cmajor=True)

    oids = np.concatenate(
        [np.asarray(tgt_ids, np.int64)[:, None],
         np.asarray(neg_ids, np.int64)], axis=1)
    slot = part[:, None] + P * (OUT_SLOTS * ltile[:, None]
                                + np.arange(NOUT)[None, :])
    out_g, out_s = build(oids, slot, plan.caps[1], plan.s_dust_out)

    return {
        "ctx_gidx": ctx_g, "ctx_sidx": ctx_s,
        "out_gidx": out_g, "out_sidx": out_s,
    }


def _build_program(plan, repeat=1):
    from contextlib import ExitStack

    import concourse.bacc as bacc
    import concourse.mybir as mybir
    import concourse.tile as tile
    from concourse.library_config import mlp as mlp_lib

    T = plan.T
    TH = plan.TH
    f32 = mybir.dt.float32
    bf16 = mybir.dt.bfloat16
    i16 = mybir.dt.int16
    AL = mybir.AluOpType
    GH = OUT_SLOTS // 2  # 6
    HC = CTX // 2        # 5

    nc = bacc.Bacc("TRN2", num_swdge_queues=NQ,
                   dynamic_dma_scratch_size=SCRATCH)

    w_embed = nc.dram_tensor("w_embed", (V, D), bf16, kind="ExternalInput")
    w_out = nc.dram_tensor("w_out", (V, D), bf16, kind="ExternalInput")
    ctx_gidx = nc.dram_tensor("ctx_gidx", (P, plan.tot[0] // 16), i16,
                              kind="ExternalInput")
    ctx_sidx = nc.dram_tensor("ctx_sidx", (P, plan.tot[0] // 16), i16,
                              kind="ExternalInput")
    out_gidx = nc.dram_tensor("out_gidx", (P, plan.tot[1] // 16), i16,
                              kind="ExternalInput")
    out_sidx = nc.dram_tensor("out_sidx", (P, plan.tot[1] // 16), i16,
                              kind="ExternalInput")
    out = nc.dram_tensor("out", (P, 1), f32, kind="ExternalOutput")

    with tile.TileContext(nc) as tc, ExitStack() as ctx:
        cpool = ctx.enter_context(tc.tile_pool(name="const", bufs=1))
        gpool = ctx.enter_context(tc.tile_pool(name="work", bufs=3))
        tpool = ctx.enter_context(tc.tile_pool(name="tree", bufs=1))

        nc.gpsimd.load_library(mlp_lib)

        cg = cpool.tile([P, plan.tot[0] // 16], i16)
        cs = cpool.tile([P, plan.tot[0] // 16], i16)
        og = cpool.tile([P, plan.tot[1] // 16], i16)
        os_ = cpool.tile([P, plan.tot[1] // 16], i16)
        g0 = plan.gn[0][0] // 16
        nc.sync.dma_start(out=cg[:, :g0], in_=ctx_gidx[:][:, :g0])
        nc.scalar.dma_start(out=og[:], in_=out_gidx[:][:, :])
        nc.sync.dma_start(out=cg[:, g0:], in_=ctx_gidx[:][:, g0:])
        nc.scalar.dma_start(out=os_[:], in_=out_sidx[:][:, :])
        nc.sync.dma_start(out=cs[:], in_=ctx_sidx[:][:, :])

        # static sign tile: +1/CTX everywhere, -1/CTX at the target slot
        sign = cpool.tile([P, NQ, 2, TH, GH], bf16)
        nc.vector.memset(sign[:], 1.0 / CTX)
        nc.vector.memset(sign[:, :, 0:1, :, 0:1], -1.0 / CTX)

        with nc.allow_low_precision("bf16 pipeline validated vs f32 ref"):
          for _rep in range(repeat):
            ce = [cpool.tile([P, plan.g_ctx, D], bf16, tag=f"ce{h}",
                             name=f"ce{h}") for h in range(NQ)]
            co = [cpool.tile([P, plan.g_ctx, D], bf16, tag=f"co{h}",
                             name=f"co{h}") for h in range(NQ)]
            oe = [cpool.tile([P, plan.g_out, D], bf16, tag=f"oe{h}",
                             name=f"oe{h}") for h in range(NQ)]
            oo = [cpool.tile([P, plan.g_out, D], bf16, tag=f"oo{h}",
                             name=f"oo{h}") for h in range(NQ)]
            for h in range(NQ):
                nc.vector.memset(ce[h][:], 0.0)
                nc.vector.memset(co[h][:], 0.0)
                nc.vector.memset(oe[h][:], 0.0)
                nc.vector.memset(oo[h][:], 0.0)

            tabs = (w_embed, w_out)
            gidx = (cg, og)
            sidx = (cs, os_)

            def emit_gather(s, w):
                n = plan.gn[s][w]
                raw = gpool.tile([P, plan.maxg // P, D], bf16, tag="raw")
                hoff = plan.goff[s][w]
                if 'gather' not in PARTS:
                    nc.vector.memset(raw[:], 0.0)
                    return raw
                o = 0
                for j, cn in enumerate(_chunks_of(n)):
                    nc.gpsimd.dma_gather(
                        out_ap=raw[:, o // P:(o + cn) // P, :],
                        in_ap=tabs[s][w * WB:min((w + 1) * WB, V), :],
                        idxs_ap=gidx[s][:, (hoff + o) // 16:
                                        (hoff + o + cn) // 16],
                        num_idxs=cn, num_idxs_reg=cn, elem_size=D,
                        single_packet=SINGLE_PACKET,
                        queue_num=(2 * w + s + j) % NQ)
                    o += cn
                return raw

            def emit_scatters(s, w, raw, only_q=None):
                if 'scatter' not in PARTS:
                    return
                hoff = plan.goff[s][w]
                for q in range(NQ):
                    if only_q is not None and q != only_q:
                        continue
                    nq = plan.caps[s][w][q]
                    qo = plan.qoff[s][w][q]
                    if s == 0:
                        eacc, oacc = ce[q], co[q]
                    else:
                        eacc, oacc = oe[q], oo[q]
                    o = 0
                    for cn in _chunks_of(nq):
                        soff = hoff + qo + o
                        ro = qo + o
                        nc.gpsimd.dma_scatter_add(
                            out_ap=eacc[:],
                            in_ap=raw[:, ro // P:(ro + cn) // P, :],
                            idxs_ap=sidx[s][:, soff // 16:(soff + cn) // 16],
                            num_idxs=cn, num_idxs_reg=cn, elem_size=D,
                            queue_num=q, sbuf_tokens_per_rank=P,
                            single_packet=SINGLE_PACKET,
                            parity_reg=0, out_ap_other=oacc[:])
                        o += cn

            united = cpool.tile([P, T, D], bf16, tag="united")
            scr = cpool.tile([P, NQ, 2, TH, GH], bf16, tag="scr")

            def emit_ctx_reduce(h):
                # ce holds even tiles' ctx sums (rank=lt), co odd tiles
                uh = united[:, h * TH:(h + 1) * TH, :].rearrange(
                    "p (a b) d -> p a b d", b=2)
                nc.vector.tensor_scalar_add(
                    out=uh[:, :, 0:1, :],
                    in0=ce[h][:, :TH // 2, :].unsqueeze(2), scalar1=0.0)
                nc.vector.tensor_scalar_add(
                    out=uh[:, :, 1:2, :],
                    in0=co[h][:, :TH // 2, :].unsqueeze(2), scalar1=0.0)

            def emit_score_reduce(h):
                se = oe[h][:, :TH * GH, :].rearrange(
                    "p (t g) d -> p t g d", g=GH)
                so = oo[h][:, :TH * GH, :].rearrange(
                    "p (t g) d -> p t g d", g=GH)
                ub = united[:, h * TH:(h + 1) * TH, :] \
                    .unsqueeze(2).broadcast_to([P, TH, GH, D])
                pr = tpool.tile([P, TH, GH, D], bf16, tag="pr", name="pr")
                nc.vector.tensor_tensor(out=pr[:], in0=se, in1=ub, op=AL.mult)
                nc.vector.tensor_reduce(
                    out=scr[:, h, 0], in_=pr[:],
                    axis=mybir.AxisListType.X, op=AL.add)
                nc.vector.tensor_tensor(out=pr[:], in0=so, in1=ub, op=AL.mult)
                nc.vector.tensor_reduce(
                    out=scr[:, h, 1], in_=pr[:],
                    axis=mybir.AxisListType.X, op=AL.add)

            do_tail = 'tail' in PARTS
            pend = None
            for w in range(NW):
                for s in range(2):
                    raw = emit_gather(s, w)
                    if pend is not None:
                        emit_scatters(*pend)
                        # ctx accumulation is complete once the last ctx
                        # group's scatters are emitted; overlap its
                        # reduction with the final out-group drain
                        if do_tail and pend[0] == 0 and pend[1] == NW - 1:
                            for h in range(NQ):
                                emit_ctx_reduce(h)
                    pend = (s, w, raw)
            # final out group: interleave each quarter's scatters with its
            # score reduction so only quarter 3's reduce is exposed
            ls, lw, lraw = pend
            for q in range(NQ):
                emit_scatters(ls, lw, lraw, only_q=q)
                if do_tail:
                    emit_score_reduce(q)

            if not do_tail:
                acc = cpool.tile([P, 1], f32, tag="acc", name="acc")
                nc.vector.memset(acc[:], 0.0)
                # touch accs so scatters stay live
                nc.vector.tensor_reduce(
                    out=acc[0:P], in_=oe[0][:].rearrange(
                        "p a b -> p (a b)").unsqueeze(1),
                    axis=mybir.AxisListType.X, op=AL.max)
                nc.sync.dma_start(out=out[:][:, :], in_=acc[:])
                continue

            # ---- softplus(sign*score) via Taylor on DVE ----
            x = cpool.tile([P, NQ, 2, TH, GH], f32, tag="x", name="x")
            nc.vector.tensor_tensor(
                out=x[:], in0=scr[:], in1=sign[:], op=AL.mult)
            a = cpool.tile([P, NQ, 2, TH, GH], f32, tag="a", name="a")
            nc.vector.tensor_tensor(out=a[:], in0=x[:], in1=x[:], op=AL.mult)
            tq = cpool.tile([P, NQ, 2, TH, GH], f32, tag="tq", name="tq")
            nc.vector.tensor_scalar(
                out=tq[:], in0=a[:], scalar1=-1.0 / 192.0, scalar2=0.125,
                op0=AL.mult, op1=AL.add)
            nc.vector.tensor_tensor(out=a[:], in0=a[:], in1=tq[:], op=AL.mult)
            nc.vector.tensor_scalar(
                out=x[:], in0=x[:], scalar1=0.5, scalar2=LN2,
                op0=AL.mult, op1=AL.add)
            acc = cpool.tile([P, 1], f32, tag="acc", name="acc")
            nc.vector.tensor_tensor(
                out=x[:], in0=x[:], in1=a[:], op=AL.add)
            xf = x[:].rearrange("p a b c d -> p (a b c d)").unsqueeze(1)
            nc.vector.tensor_reduce(
                out=acc[:], in_=xf, axis=mybir.AxisListType.X, op=AL.add)
            nc.sync.dma_start(out=out[:][:, :], in_=acc[:])

    if not nc.is_finalized():
        nc.finalize()
    return nc


def _get_program(plan, repeat=1):
    key = (plan.T, plan.caps, repeat)
    if key not in _PROGRAM_CACHE:
        _PROGRAM_CACHE[key] = _build_program(plan, repeat=repeat)
    return _PROGRAM_CACHE[key]


def _make_plan(context_ids, target_ids, neg_ids):
    B = context_ids.shape[0]
    assert B % (NCORES * P) == 0, B
    T = B // (NCORES * P)
    items = T * P
    TH = T // NQ
    ctx = np.asarray(context_ids).reshape(NCORES, items, CTX)
    oid = np.concatenate(
        [np.asarray(target_ids).reshape(NCORES, items, 1),
         np.asarray(neg_ids).reshape(NCORES, items, NOUT - 1)], axis=2)
    ctx_caps = np.zeros((NW, NQ), np.int64)
    out_caps = np.zeros((NW, NQ), np.int64)
    for c in range(NCORES):
        ctx_caps = np.maximum(ctx_caps, _qw_counts(ctx[c], items, TH))
        out_caps = np.maximum(out_caps, _qw_counts(oid[c], items, TH))
    ctx_caps = [[int(_round_up(n, P)) for n in w] for w in ctx_caps]
    out_caps = [[int(_round_up(n, P)) for n in w] for w in out_caps]
    return _Plan(T, ctx_caps, out_caps), B, T


def _prep_inputs(W_embed, W_out, context_ids, target_ids, neg_ids):
    import ml_dtypes
    plan, B, T = _make_plan(context_ids, target_ids, neg_ids)

    w_e = np.ascontiguousarray(
        np.asarray(W_embed, np.float32).astype(ml_dtypes.bfloat16))
    w_o = np.ascontiguousarray(
        np.asarray(W_out, np.float32).astype(ml_dtypes.bfloat16))
    ctx = np.asarray(context_ids).reshape(NCORES, plan.items, CTX)
    tgt = np.asarray(target_ids).reshape(NCORES, plan.items)
    neg = np.asarray(neg_ids).reshape(NCORES, plan.items, NOUT - 1)

    in_maps = []
    for c in range(NCORES):
        m = _host_prep_core(plan, ctx[c], tgt[c], neg[c])
        m["w_embed"] = w_e
        m["w_out"] = w_o
        in_maps.append(m)
    return in_maps, B, T, plan


def _run(W_embed, W_out, context_ids, target_ids, neg_ids, **spmd_kwargs):
    from concourse import bass_utils

    in_maps, B, T, plan = _prep_inputs(
        W_embed, W_out, context_ids, target_ids, neg_ids)
    nc = _get_program(plan)
    res = bass_utils.run_bass_kernel_spmd(
        nc, in_maps, core_ids=list(range(NCORES)), **spmd_kwargs)
    total = 0.0
    for r in res.results:
        total += float(r["out"].astype(np.float64).sum())
    total -= NCORES * plan.spare_per_core * LN2
    loss = np.float32(total / B)
    return loss, res


def kernel(W_embed, W_out, context_ids, target_ids, neg_ids):
    loss, _ = _run(W_embed, W_out, context_ids, target_ids, neg_ids)
    return loss
